# revision 13
# baseline (speedup 1.0000x reference)
"""Spiking autoencoder (integrate-and-fire, 16 timesteps) on 8 TRN2 NeuronCores.

Data-parallel: batch 16384 split as 8 x 2048. Per core, a fully fused
Bass/Tile kernel:

  - features are PE-transposed to feature-major [896(pad), B] layout and
    quantized to integer levels G = rne(16*x) (one fused DVE op).
  - Layer-0 spike counts have the closed form F_t = floor(G*t/16): computed
    DIRECTLY per step (no recurrence) with a fused floor-by-2^23 op, split
    between the Scalar(Act) engine (tiles 0:4, two affine ops) and the DVE
    (tiles 4:7, one fused op). F_t feeds V1_t = W1 @ F_t.
  - Layers 1-3 keep an integer spike-count state h = -#spikes:
        s_t = [h + V >= 1], h -= s_t
    and layers 2-4 rebuild potentials fresh as -W @ h (exact), so spikes
    are never materialized. Layer 4 keeps only the output count:
    C += [V4 - C >= 1].
  - W1/W2/W3 matmuls run as float32r split hi=rne11(W), lo=W-hi (two
    accumulating fp32r matmuls = fp32 precision). W4 uses hi only (12-bit
    weights) - validated to keep rel err ~8e-3, and halves decoder matmuls.
  - Layer-4 work is split into 4 feature/batch groups pipelined against
    the DVE count op; the whole step is scheduled so tensor and vector
    engines overlap (layer-4 of step t-1 fills layer-2/3 gaps of step t).

Custom DVE ops (fused, registered at import): ANT_DROP, ANT_COUNT,
ANT_DROP0, ANT_ROUND16, ANT_FLOORT.
"""
import sys
import copy
import itertools

sys.path.insert(0, "/opt/trn_rl_repo")

import numpy as np

# ----------------------------------------------------------------------------
# Custom DVE op registration
# ----------------------------------------------------------------------------
import concourse.dve_ops as dve_ops
from concourse.dve_ops import DveOp
from concourse.dve_spec import (
    Spec, Src0, Src1, One, Zero, C0, C1, C2, lower, _has_src1 as has_src1,
)
from concourse.dve_table_gen import DveOpSpec

_F = np.float32


def _register(name, spec):
    if name in dve_ops._SUB_OPCODE_FOR_NAME:
        return next(op for op in dve_ops.OPS if op.name == name)
    shas = {}
    for ver in ("v3", "v4"):
        s = DveOpSpec(name=name, opcode=0, uops=lower(spec, ver=ver),
                      rd1_en=has_src1(spec))
        shas[ver] = s.sha(ver)
    op = DveOp(name, spec, subdim=False, uops_sha=shas)
    dve_ops.OPS.append(op)
    dve_ops._SUB_OPCODE_FOR_NAME[name] = (
        dve_ops._CUSTOM_DVE_ROW_BASE + len(dve_ops.OPS) - 1)
    dve_ops.CUSTOM_DVE_SPECS[name] = spec
    assert dve_ops._SUB_OPCODE_FOR_NAME[name] < 0x20
    return op


# h' = h - ((h + V) >= 1)   (state update when V is cumulative)
ANT_DROP = _register("ANT_DROP", Spec(
    body=Src0 - ((Src0 + Src1) >= One),
    reference=lambda in0, in1, s0, s1, imm2:
        in0 - ((in0 + in1) >= 1.0).astype(_F)))

# C' = C + ((V - C) >= 1)
ANT_COUNT = _register("ANT_COUNT", Spec(
    body=Src0 + ((Src1 - Src0) >= One),
    reference=lambda in0, in1, s0, s1, imm2:
        in0 + ((in1 - in0) >= 1.0).astype(_F)))

# h' = -(V >= 1)   (first cumulative-drop step from zero state)
ANT_DROP0 = _register("ANT_DROP0", Spec(
    body=Zero - (Src0 >= One),
    reference=lambda in0, s0, s1, imm2: -((in0 >= 1.0).astype(_F))))

# out = ((x*C0 + C1) - C1) * C2  -> round-to-nearest-even via the 2^23 trick
ANT_ROUND16 = _register("ANT_ROUND16", Spec(
    body=((Src0 * C0 + C1) - C1) * C2,
    reference=lambda in0, s0, s1, imm2: (
        (np.float32(in0 * np.float32(s0)) + np.float32(s1))
        - np.float32(s1)) * np.float32(imm2)))

# out = ((x*C0 - C2) + C1) - C1 ; with C2 = 15/32, C1 = 3*2^22 this is
# floor(x*C0) exactly, for x*C0 a multiple of 1/16 in [0, 16]: x*C0 - 15/32
# lands strictly inside (floor-1/2, floor+1/2), and adding 3*2^22 rounds to
# integer (ulp 1) without ever leaving the [2^23, 2^24) binade.
ANT_FLOORT = _register("ANT_FLOORT", Spec(
    body=((Src0 * C0 - C2) + C1) - C1,
    reference=lambda in0, s0, s1, imm2: (
        (np.float32(np.float32(in0 * np.float32(s0)) - np.float32(imm2))
         + np.float32(s1)) - np.float32(s1))))

# ----------------------------------------------------------------------------
# Walrus-compat fixes (this container's neuronxcc rejects >1 sem-wait on
# many instruction structs and any wait on InstDrain; raw Bass also skips
# the pass that packs extended-inst ISA bytes).
# ----------------------------------------------------------------------------
from concourse import bass, mybir
from concourse.tile import TileContext
from concourse.vector_clock import ScopedClock
from concourse.bass_utils import run_bass_kernel_spmd
from concourse.masks import make_identity

_ctr = itertools.count()


def _build_wait_templates():
    nc = bass.Bass(target_bir_lowering=False)
    out = {}
    with nc.Block() as block, nc.semaphore("s") as s:
        for eng_name in ("sync", "vector", "scalar", "gpsimd", "tensor"):
            def _mk(e, _out=out):
                i = e.wait_ge(s, 0)
                _out[i.ins.engine] = i.ins
            getattr(block, eng_name)(_mk)
    return out


_WAIT_TEMPLATES = _build_wait_templates()


def _mk_wait(engine, w):
    wi = copy.deepcopy(_WAIT_TEMPLATES[engine])
    wi.name = f"I-waitsplit-{next(_ctr)}"
    si = wi.sync_info
    si.on_wait.clear()
    si.on_wait.append(w)
    return wi


def _fix_waits(nc, limit=1):
    n = 0
    for bb in nc.main_func.blocks:
        il = bb.instructions
        i = 0
        while i < len(il):
            ins = il[i]
            lim = 0 if type(ins).__name__ == "InstDrain" else limit
            si = ins.sync_info
            waits = list(si.on_wait) if (si and si.on_wait) else []
            if type(ins).__name__ != "InstEventSemaphore" and len(waits) > lim:
                keep, extra = waits[:lim], waits[lim:]
                si.on_wait.clear()
                for w in keep:
                    si.on_wait.append(w)
                for j, w in enumerate(extra):
                    il.insert(i + j, _mk_wait(ins.engine, w))
                i += len(extra)
                n += 1
            i += 1
    return n


def _finalize(nc):
    from concourse.library_overlay import lower_extended_insts
    lower_extended_insts(nc)
    return _fix_waits(nc)


def _patched_drain_and_barrier(self, tick_clock, wait_clock):
    nc = self.nc
    probe = nc.sync.nop()
    wait_clock.add_sem_waits(probe.ins, ScopedClock({None: tick_clock.global_clock}))
    si = probe.ins.sync_info
    waits = list(si.on_wait or []) if si is not None else []
    if si is not None and si.on_wait:
        si.on_wait.clear()
    handles = list(self.sems.allocated().values())
    by_name = {getattr(h, "name", None): h for h in handles}
    for w in waits:
        nc.sync.wait_ge(by_name[w.ant_name], w.wait_value)
    nc.sync.drain()
    nc.all_engine_barrier()
    popped = nc._tile_sem_poison_stack.pop()
    assert popped is self._sem_poison
    nc.clear_and_free_semaphores(handles)
    nc.all_engine_barrier()


TileContext._drain_and_barrier = _patched_drain_and_barrier

# ----------------------------------------------------------------------------
# Kernel build
# ----------------------------------------------------------------------------
F32 = mybir.dt.float32
F32R = mybir.dt.float32r
F16 = mybir.dt.float16

NCORES = 8
B = 16384
BL = B // NCORES          # 2048 per core
IN = 784
H = 128
T = 16
FT = 7                    # feature tiles
F = FT * 128              # 896 padded
BC = 512                  # batch chunk (psum-bank limited)
NCH = BL // BC            # 4 chunks
NBT = BC // 128           # 4 batch subtiles per chunk
HB = BC // 2              # 256: layer-4 half-batch group width

C23 = float(2 ** 23)
M32 = float(3 * 2 ** 22)   # 12582912, exact
OFF = 15.0 / 32.0          # 0.46875, exact

_CACHE = {}

IDENT = mybir.ActivationFunctionType.Identity
ACOPY = mybir.ActivationFunctionType.Copy
ALU = mybir.AluOpType


def _build():
    if "nc" in _CACHE:
        return _CACHE["nc"]
    nc = bass.Bass(target_bir_lowering=False)
    x_ext = nc.declare_dram_parameter("x", [BL, IN], F32, isOutput=False)
    w_ext = nc.declare_dram_parameter("wts", [128, 3200], F16, isOutput=False)
    sc_ext = nc.declare_dram_parameter("sc", [1, 1], F32, isOutput=False)
    o_ext = nc.declare_dram_parameter("out", [BL, IN], F32, isOutput=True)

    with TileContext(nc) as tc:
        with (tc.tile_pool(name="const", bufs=1) as constp,
              tc.tile_pool(name="sb", bufs=2) as sb,
              tc.tile_pool(name="st", bufs=1) as st,
              tc.tile_pool(name="st2", bufs=2) as st2,
              tc.tile_pool(name="ps", bufs=1, space="PSUM") as ps):

            wts = constp.tile([128, 3200], F16, tag="wts")
            ident = constp.tile([128, 128], F32, tag="ident")
            scb = constp.tile([128, 1], F32, tag="scb")
            make_identity(nc, ident[:])

            # wts layout: [ hi W1T 0:896 | hi W2 896:1024 | hi W3 1024:1152 |
            #               W4T(hi only) 1152:2048 |
            #               lo W1T 2048:2944 | lo W2 2944:3072 | lo W3 3072:3200 ]
            w1s = ([wts[:, k * 128:(k + 1) * 128] for k in range(FT)],
                   [wts[:, 2048 + k * 128:2048 + (k + 1) * 128] for k in range(FT)])
            w2s = (wts[:, 896:1024], wts[:, 2944:3072])
            w3s = (wts[:, 1024:1152], wts[:, 3072:3200])
            w4s = [wts[:, 1152 + j * 128:1152 + (j + 1) * 128] for j in range(FT)]

            ya = st.tile([128, 4, BC], F32, tag="ya", name="ya")
            yv = st.tile([128, 3, BC], F16, tag="yv", name="yv")

            # per-chunk state tiles (tag rotation order == creation order)
            ch = []
            for c in range(NCH):
                ch.append({
                    "G": st2.tile([128, FT, BC], F16, tag="G", name=f"G_{c}"),
                    "Fr": [st.tile([128, FT, BC], F16, tag=f"Fr{i}",
                                   name=f"Fr{i}_{c}") for i in range(3)],
                    "h1": st.tile([128, BC], F16, tag="h1", name=f"h1_{c}"),
                    "h2": st.tile([128, BC], F16, tag="h2", name=f"h2_{c}"),
                    "h3b": [st2.tile([128, BC], F16, tag=f"h3{i}",
                                     name=f"h3{i}_{c}") for i in range(3)],
                    "C": st2.tile([128, FT, BC], F32, tag="C", name=f"C_{c}"),
                })

            def vx_tile(name):
                return ps.tile([128, BC], F32, tag="Vx", bufs=2, name=name)

            def emit_v1(s2):
                """V1 block for global step s2 (one step ahead of the h-chain)."""
                c2, t2 = (s2 - 1) // T, (s2 - 1) % T + 1
                Ft = ch[c2]["Fr"][t2 % 3] if t2 < T else ch[c2]["G"]
                v = vx_tile(f"V1_{s2}")
                for k in range(FT):
                    for h in range(2):
                        nc.tensor.matmul(v[:], w1s[h][k], Ft[:, k, :],
                                         start=(k == 0 and h == 0),
                                         stop=(k == FT - 1 and h == 1))
                return v

            def in_path_b(c, b, dve_quant=False):
                """DMA + transpose + quantize one 128-row batch subtile."""
                G = ch[c]["G"]
                xt = sb.tile([128, IN], F32, tag="x", name=f"x_{c}_{b}")
                nc.sync.dma_start(
                    out=xt[:],
                    in_=x_ext[c * BC + b * 128:c * BC + (b + 1) * 128, :])
                xpsB = ps.tile([128, 3, 128], F32, tag="xt", bufs=2,
                               name=f"xpsB_{c}_{b}")
                nc.vector.memset(xpsB[:, 2, :], 0.0)
                for j in range(4, 6):
                    nc.tensor.transpose(
                        xpsB[:, j - 4, :], xt[:, j * 128:(j + 1) * 128],
                        identity=ident[:])
                nc.tensor.transpose(
                    xpsB[0:16, 2, :], xt[:, 768:784], identity=ident[:])
                yqB = sb.tile([128, 3, 128], F32, tag="yq2", name=f"yqB_{c}_{b}")
                if dve_quant:
                    nc.vector.tensor_scalar(out=yqB[:], in0=xpsB[:, :, :],
                                            scalar1=16.0, scalar2=M32,
                                            op0=ALU.mult, op1=ALU.add)
                    nc.vector.tensor_scalar(
                        out=G[:, 4:7, b * 128:(b + 1) * 128], in0=yqB[:],
                        scalar1=M32, scalar2=None, op0=ALU.subtract)
                else:
                    nc.scalar.activation(yqB[:], xpsB[:, :, :], ACOPY,
                                         bias=M32, scale=16.0)
                    nc.scalar.activation(G[:, 4:7, b * 128:(b + 1) * 128],
                                         yqB[:], ACOPY, bias=-M32, scale=1.0)
                xpsA = ps.tile([128, 4, 128], F32, tag="xt", bufs=2,
                               name=f"xpsA_{c}_{b}")
                for j in range(4):
                    nc.tensor.transpose(
                        xpsA[:, j, :], xt[:, j * 128:(j + 1) * 128],
                        identity=ident[:])
                yqA = sb.tile([128, 4, 128], F32, tag="yq", name=f"yqA_{c}_{b}")
                if dve_quant:
                    nc.vector.tensor_scalar(out=yqA[:], in0=xpsA[:, :, :],
                                            scalar1=16.0, scalar2=M32,
                                            op0=ALU.mult, op1=ALU.add)
                    nc.vector.tensor_scalar(
                        out=G[:, 0:4, b * 128:(b + 1) * 128], in0=yqA[:],
                        scalar1=M32, scalar2=None, op0=ALU.subtract)
                else:
                    nc.scalar.activation(yqA[:], xpsA[:, :, :], ACOPY,
                                         bias=M32, scale=16.0)
                    nc.scalar.activation(G[:, 0:4, b * 128:(b + 1) * 128],
                                         yqA[:], ACOPY, bias=-M32, scale=1.0)

            def floort_dve(c, t):
                """F_t tiles 4:7 on DVE (two builtin tensor_scalar ops -
                builtins hit the fast DVE perf modes, customs do not)."""
                Fo = ch[c]["Fr"][t % 3]
                nc.vector.tensor_scalar(
                    out=yv[:], in0=ch[c]["G"][:, 4:7, :],
                    scalar1=float(t) / 16.0, scalar2=OFF,
                    op0=ALU.mult, op1=ALU.subtract)
                nc.vector.tensor_scalar(
                    out=Fo[:, 4:7, :], in0=yv[:],
                    scalar1=M32, scalar2=M32, op0=ALU.add, op1=ALU.subtract)

            def floort_act(c, t):
                """F_t tiles 0:4 on Act (three exact affine Copy ops)."""
                Fo = ch[c]["Fr"][t % 3]
                nc.scalar.activation(ya[:], ch[c]["G"][:, 0:4, :], ACOPY,
                                     bias=-OFF, scale=float(t) / 16.0)
                nc.scalar.activation(ya[:], ya[:], ACOPY, bias=M32, scale=1.0)
                nc.scalar.activation(Fo[:, 0:4, :], ya[:], ACOPY,
                                     bias=-M32, scale=1.0)

            def floort_sched_dve(c, t):
                if t <= T - 3:
                    floort_dve(c, t + 2)
                elif t == T - 1 and c + 1 < NCH:
                    floort_dve(c + 1, 1)
                elif t == T and c + 1 < NCH:
                    floort_dve(c + 1, 2)

            def floort_sched_act(c, t):
                if t <= T - 3:
                    floort_act(c, t + 2)
                elif t == T - 1 and c + 1 < NCH:
                    floort_act(c + 1, 1)
                elif t == T and c + 1 < NCH:
                    floort_act(c + 1, 2)

            def d_half(c, tp, half):
                """Layer-4 potentials for (chunk c, step tp), one batch half."""
                h3 = ch[c]["h3b"][tp % 3]
                lo = half * HB
                dps = ps.tile([128, FT, HB], F32, tag="Dh", bufs=1,
                              name=f"Dh_{c}_{tp}_{half}")
                for j in range(FT):
                    nc.tensor.matmul(dps[:, j, :], w4s[j], h3[:, lo:lo + HB],
                                     start=True, stop=True)
                return dps

            def cnt(c, dps, half):
                C = ch[c]["C"]
                lo = half * HB
                nc.vector._custom_dve(
                    ANT_COUNT, out=C[:, :, lo:lo + HB],
                    in0=C[:, :, lo:lo + HB], in1=dps[:])

            def out_path_b(c, b):
                C = ch[c]["C"]
                cpsA = ps.tile([128, 4, 128], F32, tag="xt", bufs=2,
                               name=f"coA_{c}_{b}")
                for j in range(4):
                    nc.tensor.transpose(
                        cpsA[:, j, :], C[:, j, b * 128:(b + 1) * 128],
                        identity=ident[:])
                yoA = sb.tile([128, 4, 128], F32, tag="yo", name=f"yoA_{c}_{b}")
                nc.scalar.mul(yoA[:], cpsA[:], scb[:])
                cpsB = ps.tile([128, 3, 128], F32, tag="xt", bufs=2,
                               name=f"coB_{c}_{b}")
                for j in range(4, 7):
                    nc.tensor.transpose(
                        cpsB[:, j - 4, :], C[:, j, b * 128:(b + 1) * 128],
                        identity=ident[:])
                yoB = sb.tile([128, 3, 128], F32, tag="yo2", name=f"yoB_{c}_{b}")
                nc.scalar.mul(yoB[:], cpsB[:], scb[:])
                orows = slice(c * BC + b * 128, c * BC + (b + 1) * 128)
                for j in range(4):
                    nc.gpsimd.dma_start(
                        out=o_ext[orows, j * 128:(j + 1) * 128],
                        in_=yoA[:, j, :])
                for j in range(4, 6):
                    nc.gpsimd.dma_start(
                        out=o_ext[orows, j * 128:(j + 1) * 128],
                        in_=yoB[:, j - 4, :])
                nc.gpsimd.dma_start(out=o_ext[orows, 768:784],
                                    in_=yoB[:, 2, 0:16])

            # ---- prelude: chunk 0 input + first two F planes ----
            # (x DMAs queue ahead of the big wts DMA on the sync engine)
            for b in range(NBT):
                in_path_b(0, b, dve_quant=(b < 2))
            nc.sync.dma_start(out=wts[:], in_=w_ext[:])
            nc.sync.dma_start(out=scb[:], in_=sc_ext[:].to_broadcast([128, 1]))
            nc.scalar.mul(scb[:], scb[:], 1.0 / T)
            nc.gpsimd.memset(ch[0]["C"][:], 0.0)
            floort_dve(0, 1)
            floort_act(0, 1)
            floort_dve(0, 2)
            floort_act(0, 2)

            # ---- uniform software-pipelined loop over all 64 global steps:
            # V1 runs one step AHEAD (so dh1 never waits on it), layer-4/CNT
            # run two steps BEHIND (slack on the Dh psum WAR). ----
            V1cur = emit_v1(1)
            for s in range(1, NCH * T + 1):
                c, t = (s - 1) // T, (s - 1) % T + 1
                cp, tp = (s - 3) // T, (s - 3) % T + 1   # global step s-2 (lag 2)
                h1, h2, h3b = ch[c]["h1"], ch[c]["h2"], ch[c]["h3b"]
                # vector: F two steps ahead
                floort_sched_dve(c, t)
                # vector: dh1 (V1 for this step finished last step)
                if t == 1:
                    nc.vector._custom_dve(ANT_DROP0, out=h1[:], in0=V1cur[:])
                else:
                    nc.vector._custom_dve(ANT_DROP, out=h1[:], in0=h1[:],
                                          in1=V1cur[:])
                # layer-4 of global step s-2, half 0
                if s >= 3:
                    d0 = d_half(cp, tp, 0)
                    cnt(cp, d0, 0)
                # tensor: V2 = -W2 @ h1 (hi+lo)
                V2t = vx_tile(f"V2_{s}")
                for h in range(2):
                    nc.tensor.matmul(V2t[:], w2s[h], h1[:],
                                     start=(h == 0), stop=(h == 1))
                if t == 1:
                    nc.vector._custom_dve(ANT_DROP0, out=h2[:], in0=V2t[:])
                else:
                    nc.vector._custom_dve(ANT_DROP, out=h2[:], in0=h2[:],
                                          in1=V2t[:])
                if s >= 3:
                    d1 = d_half(cp, tp, 1)
                    cnt(cp, d1, 1)
                # tensor: V3 = -W3 @ h2 (hi+lo)
                V3t = vx_tile(f"V3_{s}")
                for h in range(2):
                    nc.tensor.matmul(V3t[:], w3s[h], h2[:],
                                     start=(h == 0), stop=(h == 1))
                if t == 1:
                    nc.vector._custom_dve(ANT_DROP0, out=h3b[1][:], in0=V3t[:])
                else:
                    nc.vector._custom_dve(ANT_DROP, out=h3b[t % 3][:],
                                          in0=h3b[(t - 1) % 3][:], in1=V3t[:])
                # tensor: V1 for the NEXT global step
                if s < NCH * T:
                    V1cur = emit_v1(s + 1)
                # scalar: F two steps ahead (Act part)
                floort_sched_act(c, t)
                # interleaved chunk bookkeeping (step tails)
                if T - 5 <= t <= T - 2 and c + 1 < NCH:
                    in_path_b(c + 1, t - (T - 5))
                if t == T - 1 and c + 1 < NCH:
                    nc.gpsimd.memset(ch[c + 1]["C"][:], 0.0)
                if 3 <= t <= 6 and c >= 1:
                    out_path_b(c - 1, t - 3)

            # ---- postlude: layer 4 of the last two global steps + out ----
            d0 = d_half(NCH - 1, T - 1, 0)
            cnt(NCH - 1, d0, 0)
            d1 = d_half(NCH - 1, T - 1, 1)
            cnt(NCH - 1, d1, 1)
            d0 = d_half(NCH - 1, T, 0)
            cnt(NCH - 1, d0, 0)
            d1 = d_half(NCH - 1, T, 1)
            out_path_b(NCH - 1, 0)
            out_path_b(NCH - 1, 1)
            cnt(NCH - 1, d1, 1)
            out_path_b(NCH - 1, 2)
            out_path_b(NCH - 1, 3)

    _finalize(nc)
    _CACHE["nc"] = nc
    return nc


def _rne11(x):
    xi = np.asarray(x, np.float32).view(np.uint32).astype(np.uint64)
    half = np.uint64(1 << 11)
    lsb = (xi >> np.uint64(12)) & np.uint64(1)
    q = ((xi + half - np.uint64(1) + lsb) >> np.uint64(12)) << np.uint64(12)
    return np.minimum(q, np.uint64(0xFFFFFFFF)).astype(np.uint32).view(np.float32)


def _prep_inputs(features, W1, W2, W3, W4, out_scale):
    f32 = np.float32
    W1p = np.zeros((H, F), f32); W1p[:, :IN] = W1
    W4p = np.zeros((F, H), f32); W4p[:IN, :] = W4
    W1T = W1p.T.reshape(FT, 128, H).transpose(1, 0, 2).reshape(128, FT * H)
    w123 = np.concatenate([W1T, -W2.T.astype(f32), -W3.T.astype(f32)], axis=1)
    hi = w123.astype(np.float16)
    lo = (w123 - hi.astype(f32)).astype(np.float16)
    w4hi = (-W4p.T).astype(np.float16)
    wts = np.ascontiguousarray(
        np.concatenate([hi, w4hi, lo], axis=1), dtype=np.float16)
    assert wts.shape == (128, 3200)
    sc = np.asarray(out_scale, f32).reshape(1, 1)
    in_maps = []
    for i in range(NCORES):
        in_maps.append({
            "x": np.ascontiguousarray(features[i * BL:(i + 1) * BL], f32),
            "wts": wts,
            "sc": sc,
        })
    return in_maps


def _run(inputs, trace=False):
    nc = _build()
    in_maps = _prep_inputs(**inputs)
    res = run_bass_kernel_spmd(nc, in_maps, core_ids=list(range(NCORES)),
                               trace=trace)
    out = np.concatenate([res.results[i]["out"] for i in range(NCORES)], axis=0)
    return out.astype(np.float32), res


def kernel(**inputs):
    out, _ = _run(inputs, trace=False)
    return out


# revision 14
# speedup vs baseline: 1.1021x; 1.1021x over previous
"""Spiking autoencoder (integrate-and-fire, 16 timesteps) on 8 TRN2 NeuronCores.

Data-parallel: batch 16384 split as 8 x 2048. Per core, a fully fused
Bass/Tile kernel:

  - features are PE-transposed to feature-major [896(pad), B] layout and
    quantized to integer levels G = rne(16*x) (one fused DVE op).
  - Layer-0 spike counts have the closed form F_t = floor(G*t/16): computed
    DIRECTLY per step (no recurrence) with a fused floor-by-2^23 op, split
    between the Scalar(Act) engine (tiles 0:4, two affine ops) and the DVE
    (tiles 4:7, one fused op). F_t feeds V1_t = W1 @ F_t.
  - Layers 1-3 keep an integer spike-count state h = -#spikes:
        s_t = [h + V >= 1], h -= s_t
    and layers 2-4 rebuild potentials fresh as -W @ h (exact), so spikes
    are never materialized. Layer 4 keeps only the output count:
    C += [V4 - C >= 1].
  - W1/W2/W3 matmuls run as float32r split hi=rne11(W), lo=W-hi (two
    accumulating fp32r matmuls = fp32 precision). W4 uses hi only (12-bit
    weights) - validated to keep rel err ~8e-3, and halves decoder matmuls.
  - Layer-4 work is split into 4 feature/batch groups pipelined against
    the DVE count op; the whole step is scheduled so tensor and vector
    engines overlap (layer-4 of step t-1 fills layer-2/3 gaps of step t).

Custom DVE ops (fused, registered at import): ANT_DROP, ANT_COUNT,
ANT_DROP0, ANT_ROUND16, ANT_FLOORT.
"""
import sys
import copy
import itertools

sys.path.insert(0, "/opt/trn_rl_repo")

import numpy as np

# ----------------------------------------------------------------------------
# Custom DVE op registration
# ----------------------------------------------------------------------------
import concourse.dve_ops as dve_ops
from concourse.dve_ops import DveOp
from concourse.dve_spec import (
    Spec, Src0, Src1, One, Zero, C0, C1, C2, lower, _has_src1 as has_src1,
)
from concourse.dve_table_gen import DveOpSpec

_F = np.float32


def _register(name, spec):
    if name in dve_ops._SUB_OPCODE_FOR_NAME:
        return next(op for op in dve_ops.OPS if op.name == name)
    shas = {}
    for ver in ("v3", "v4"):
        s = DveOpSpec(name=name, opcode=0, uops=lower(spec, ver=ver),
                      rd1_en=has_src1(spec))
        shas[ver] = s.sha(ver)
    op = DveOp(name, spec, subdim=False, uops_sha=shas)
    dve_ops.OPS.append(op)
    dve_ops._SUB_OPCODE_FOR_NAME[name] = (
        dve_ops._CUSTOM_DVE_ROW_BASE + len(dve_ops.OPS) - 1)
    dve_ops.CUSTOM_DVE_SPECS[name] = spec
    assert dve_ops._SUB_OPCODE_FOR_NAME[name] < 0x20
    return op


# h' = h - ((h + V) >= 1)   (state update when V is cumulative)
ANT_DROP = _register("ANT_DROP", Spec(
    body=Src0 - ((Src0 + Src1) >= One),
    reference=lambda in0, in1, s0, s1, imm2:
        in0 - ((in0 + in1) >= 1.0).astype(_F)))

# C' = C + ((V - C) >= 1)
ANT_COUNT = _register("ANT_COUNT", Spec(
    body=Src0 + ((Src1 - Src0) >= One),
    reference=lambda in0, in1, s0, s1, imm2:
        in0 + ((in1 - in0) >= 1.0).astype(_F)))

# h' = -(V >= 1)   (first cumulative-drop step from zero state)
ANT_DROP0 = _register("ANT_DROP0", Spec(
    body=Zero - (Src0 >= One),
    reference=lambda in0, s0, s1, imm2: -((in0 >= 1.0).astype(_F))))

# out = ((x*C0 + C1) - C1) * C2  -> round-to-nearest-even via the 2^23 trick
ANT_ROUND16 = _register("ANT_ROUND16", Spec(
    body=((Src0 * C0 + C1) - C1) * C2,
    reference=lambda in0, s0, s1, imm2: (
        (np.float32(in0 * np.float32(s0)) + np.float32(s1))
        - np.float32(s1)) * np.float32(imm2)))

# out = ((x*C0 - C2) + C1) - C1 ; with C2 = 15/32, C1 = 3*2^22 this is
# floor(x*C0) exactly, for x*C0 a multiple of 1/16 in [0, 16]: x*C0 - 15/32
# lands strictly inside (floor-1/2, floor+1/2), and adding 3*2^22 rounds to
# integer (ulp 1) without ever leaving the [2^23, 2^24) binade.
ANT_FLOORT = _register("ANT_FLOORT", Spec(
    body=((Src0 * C0 - C2) + C1) - C1,
    reference=lambda in0, s0, s1, imm2: (
        (np.float32(np.float32(in0 * np.float32(s0)) - np.float32(imm2))
         + np.float32(s1)) - np.float32(s1))))

# ----------------------------------------------------------------------------
# Walrus-compat fixes (this container's neuronxcc rejects >1 sem-wait on
# many instruction structs and any wait on InstDrain; raw Bass also skips
# the pass that packs extended-inst ISA bytes).
# ----------------------------------------------------------------------------
from concourse import bass, mybir
from concourse.tile import TileContext
from concourse.vector_clock import ScopedClock
from concourse.bass_utils import run_bass_kernel_spmd
from concourse.masks import make_identity

_ctr = itertools.count()


def _build_wait_templates():
    nc = bass.Bass(target_bir_lowering=False)
    out = {}
    with nc.Block() as block, nc.semaphore("s") as s:
        for eng_name in ("sync", "vector", "scalar", "gpsimd", "tensor"):
            def _mk(e, _out=out):
                i = e.wait_ge(s, 0)
                _out[i.ins.engine] = i.ins
            getattr(block, eng_name)(_mk)
    return out


_WAIT_TEMPLATES = _build_wait_templates()


def _mk_wait(engine, w):
    wi = copy.deepcopy(_WAIT_TEMPLATES[engine])
    wi.name = f"I-waitsplit-{next(_ctr)}"
    si = wi.sync_info
    si.on_wait.clear()
    si.on_wait.append(w)
    return wi


def _fix_waits(nc, limit=1):
    n = 0
    for bb in nc.main_func.blocks:
        il = bb.instructions
        i = 0
        while i < len(il):
            ins = il[i]
            lim = 0 if type(ins).__name__ == "InstDrain" else limit
            si = ins.sync_info
            waits = list(si.on_wait) if (si and si.on_wait) else []
            if type(ins).__name__ != "InstEventSemaphore" and len(waits) > lim:
                keep, extra = waits[:lim], waits[lim:]
                si.on_wait.clear()
                for w in keep:
                    si.on_wait.append(w)
                for j, w in enumerate(extra):
                    il.insert(i + j, _mk_wait(ins.engine, w))
                i += len(extra)
                n += 1
            i += 1
    return n


def _finalize(nc):
    from concourse.library_overlay import lower_extended_insts
    lower_extended_insts(nc)
    return _fix_waits(nc)


def _patched_drain_and_barrier(self, tick_clock, wait_clock):
    nc = self.nc
    probe = nc.sync.nop()
    wait_clock.add_sem_waits(probe.ins, ScopedClock({None: tick_clock.global_clock}))
    si = probe.ins.sync_info
    waits = list(si.on_wait or []) if si is not None else []
    if si is not None and si.on_wait:
        si.on_wait.clear()
    handles = list(self.sems.allocated().values())
    by_name = {getattr(h, "name", None): h for h in handles}
    for w in waits:
        nc.sync.wait_ge(by_name[w.ant_name], w.wait_value)
    nc.sync.drain()
    nc.all_engine_barrier()
    popped = nc._tile_sem_poison_stack.pop()
    assert popped is self._sem_poison
    nc.clear_and_free_semaphores(handles)
    nc.all_engine_barrier()


TileContext._drain_and_barrier = _patched_drain_and_barrier

# ----------------------------------------------------------------------------
# Kernel build
# ----------------------------------------------------------------------------
F32 = mybir.dt.float32
F32R = mybir.dt.float32r
F16 = mybir.dt.float16

NCORES = 8
B = 16384
BL = B // NCORES          # 2048 per core
IN = 784
H = 128
T = 16
FT = 7                    # feature tiles
F = FT * 128              # 896 padded
BC = 512                  # batch chunk (psum-bank limited)
NCH = BL // BC            # 4 chunks
NBT = BC // 128           # 4 batch subtiles per chunk
HB = BC // 2              # 256: layer-4 half-batch group width

C23 = float(2 ** 23)
M32 = float(3 * 2 ** 22)   # 12582912, exact
OFF = 15.0 / 32.0          # 0.46875, exact

_CACHE = {}

IDENT = mybir.ActivationFunctionType.Identity
ACOPY = mybir.ActivationFunctionType.Copy
ALU = mybir.AluOpType


def _build():
    if "nc" in _CACHE:
        return _CACHE["nc"]
    nc = bass.Bass(target_bir_lowering=False)
    x_ext = nc.declare_dram_parameter("x", [BL, IN], F32, isOutput=False)
    w_ext = nc.declare_dram_parameter("wts", [128, 3200], F16, isOutput=False)
    sc_ext = nc.declare_dram_parameter("sc", [1, 1], F32, isOutput=False)
    o_ext = nc.declare_dram_parameter("out", [BL, IN], F32, isOutput=True)

    with TileContext(nc) as tc:
        with (tc.tile_pool(name="const", bufs=1) as constp,
              tc.tile_pool(name="sb", bufs=2) as sb,
              tc.tile_pool(name="st", bufs=1) as st,
              tc.tile_pool(name="st2", bufs=2) as st2,
              tc.tile_pool(name="ps", bufs=1, space="PSUM") as ps):

            wts = constp.tile([128, 3200], F16, tag="wts")
            ident = constp.tile([128, 128], F32, tag="ident")
            scb = constp.tile([128, 1], F32, tag="scb")
            make_identity(nc, ident[:])

            # wts layout: [ hi W1T 0:896 | hi W2 896:1024 | hi W3 1024:1152 |
            #               W4T(hi only) 1152:2048 |
            #               lo W1T 2048:2944 | lo W2 2944:3072 | lo W3 3072:3200 ]
            w1s = ([wts[:, k * 128:(k + 1) * 128] for k in range(FT)],
                   [wts[:, 2048 + k * 128:2048 + (k + 1) * 128] for k in range(FT)])
            w2s = (wts[:, 896:1024], wts[:, 2944:3072])
            w3s = (wts[:, 1024:1152], wts[:, 3072:3200])
            w4s = [wts[:, 1152 + j * 128:1152 + (j + 1) * 128] for j in range(FT)]

            ya = st.tile([128, 4, BC], F32, tag="ya", name="ya")
            yv = st.tile([128, 3, BC], F16, tag="yv", name="yv")

            # per-chunk state tiles (tag rotation order == creation order)
            ch = []
            for c in range(NCH):
                ch.append({
                    "G": st2.tile([128, FT, BC], F16, tag="G", name=f"G_{c}"),
                    "Fr": [st.tile([128, FT, BC], F16, tag=f"Fr{i}",
                                   name=f"Fr{i}_{c}") for i in range(3)],
                    "h1": st.tile([128, BC], F16, tag="h1", name=f"h1_{c}"),
                    "h2": st.tile([128, BC], F16, tag="h2", name=f"h2_{c}"),
                    "h3b": [st2.tile([128, BC], F16, tag=f"h3{i}",
                                     name=f"h3{i}_{c}") for i in range(3)],
                    "C": st2.tile([128, FT, BC], F32, tag="C", name=f"C_{c}"),
                })

            V23ps = ps.tile([128, BC], F32, tag="V23", name="V23ps")

            def emit_v1(s2):
                """V1 block for global step s2 (one step ahead of the h-chain)."""
                c2, t2 = (s2 - 1) // T, (s2 - 1) % T + 1
                Ft = ch[c2]["Fr"][t2 % 3] if t2 < T else ch[c2]["G"]
                v = ps.tile([128, BC], F32, tag="V1", bufs=2, name=f"V1_{s2}")
                for k in range(FT):
                    for h in range(2):
                        nc.tensor.matmul(v[:], w1s[h][k], Ft[:, k, :],
                                         start=(k == 0 and h == 0),
                                         stop=(k == FT - 1 and h == 1))
                return v

            def in_path_b(c, b, dve_quant=False):
                """DMA + transpose + quantize one 128-row batch subtile."""
                G = ch[c]["G"]
                xt = sb.tile([128, IN], F32, tag="x", name=f"x_{c}_{b}")
                nc.sync.dma_start(
                    out=xt[:],
                    in_=x_ext[c * BC + b * 128:c * BC + (b + 1) * 128, :])
                xpsB = ps.tile([128, 3, 128], F32, tag="xt", bufs=1,
                               name=f"xpsB_{c}_{b}")
                nc.vector.memset(xpsB[:, 2, :], 0.0)
                for j in range(4, 6):
                    nc.tensor.transpose(
                        xpsB[:, j - 4, :], xt[:, j * 128:(j + 1) * 128],
                        identity=ident[:])
                nc.tensor.transpose(
                    xpsB[0:16, 2, :], xt[:, 768:784], identity=ident[:])
                yqB = sb.tile([128, 3, 128], F32, tag="yq2", name=f"yqB_{c}_{b}")
                if dve_quant:
                    nc.vector.tensor_scalar(out=yqB[:], in0=xpsB[:, :, :],
                                            scalar1=16.0, scalar2=M32,
                                            op0=ALU.mult, op1=ALU.add)
                    nc.vector.tensor_scalar(
                        out=G[:, 4:7, b * 128:(b + 1) * 128], in0=yqB[:],
                        scalar1=M32, scalar2=None, op0=ALU.subtract)
                else:
                    nc.scalar.activation(yqB[:], xpsB[:, :, :], ACOPY,
                                         bias=M32, scale=16.0)
                    nc.scalar.activation(G[:, 4:7, b * 128:(b + 1) * 128],
                                         yqB[:], ACOPY, bias=-M32, scale=1.0)
                xpsA = ps.tile([128, 4, 128], F32, tag="xt", bufs=1,
                               name=f"xpsA_{c}_{b}")
                for j in range(4):
                    nc.tensor.transpose(
                        xpsA[:, j, :], xt[:, j * 128:(j + 1) * 128],
                        identity=ident[:])
                yqA = sb.tile([128, 4, 128], F32, tag="yq", name=f"yqA_{c}_{b}")
                if dve_quant:
                    nc.vector.tensor_scalar(out=yqA[:], in0=xpsA[:, :, :],
                                            scalar1=16.0, scalar2=M32,
                                            op0=ALU.mult, op1=ALU.add)
                    nc.vector.tensor_scalar(
                        out=G[:, 0:4, b * 128:(b + 1) * 128], in0=yqA[:],
                        scalar1=M32, scalar2=None, op0=ALU.subtract)
                else:
                    nc.scalar.activation(yqA[:], xpsA[:, :, :], ACOPY,
                                         bias=M32, scale=16.0)
                    nc.scalar.activation(G[:, 0:4, b * 128:(b + 1) * 128],
                                         yqA[:], ACOPY, bias=-M32, scale=1.0)

            def floort_dve(c, t):
                """F_t tiles 4:7 on DVE (two builtin tensor_scalar ops -
                builtins hit the fast DVE perf modes, customs do not)."""
                Fo = ch[c]["Fr"][t % 3]
                nc.vector.tensor_scalar(
                    out=yv[:], in0=ch[c]["G"][:, 4:7, :],
                    scalar1=float(t) / 16.0, scalar2=OFF,
                    op0=ALU.mult, op1=ALU.subtract)
                nc.vector.tensor_scalar(
                    out=Fo[:, 4:7, :], in0=yv[:],
                    scalar1=M32, scalar2=M32, op0=ALU.add, op1=ALU.subtract)

            def floort_act(c, t):
                """F_t tiles 0:4 on Act (three exact affine Copy ops)."""
                Fo = ch[c]["Fr"][t % 3]
                nc.scalar.activation(ya[:], ch[c]["G"][:, 0:4, :], ACOPY,
                                     bias=-OFF, scale=float(t) / 16.0)
                nc.scalar.activation(ya[:], ya[:], ACOPY, bias=M32, scale=1.0)
                nc.scalar.activation(Fo[:, 0:4, :], ya[:], ACOPY,
                                     bias=-M32, scale=1.0)

            def floort_sched_dve(c, t):
                if t <= T - 3:
                    floort_dve(c, t + 2)
                elif t == T - 1 and c + 1 < NCH:
                    floort_dve(c + 1, 1)
                elif t == T and c + 1 < NCH:
                    floort_dve(c + 1, 2)

            def floort_sched_act(c, t):
                if t <= T - 3:
                    floort_act(c, t + 2)
                elif t == T - 1 and c + 1 < NCH:
                    floort_act(c + 1, 1)
                elif t == T and c + 1 < NCH:
                    floort_act(c + 1, 2)

            def d_half(c, tp, half):
                """Layer-4 potentials for (chunk c, step tp), one batch half."""
                h3 = ch[c]["h3b"][tp % 3]
                lo = half * HB
                dps = ps.tile([128, FT, HB], F32, tag="Dh", bufs=1,
                              name=f"Dh_{c}_{tp}_{half}")
                for j in range(FT):
                    nc.tensor.matmul(dps[:, j, :], w4s[j], h3[:, lo:lo + HB],
                                     start=True, stop=True)
                return dps

            def cnt(c, dps, half):
                C = ch[c]["C"]
                lo = half * HB
                nc.vector._custom_dve(
                    ANT_COUNT, out=C[:, :, lo:lo + HB],
                    in0=C[:, :, lo:lo + HB], in1=dps[:])

            def out_path_b(c, b):
                C = ch[c]["C"]
                cpsA = ps.tile([128, 4, 128], F32, tag="xt", bufs=1,
                               name=f"coA_{c}_{b}")
                for j in range(4):
                    nc.tensor.transpose(
                        cpsA[:, j, :], C[:, j, b * 128:(b + 1) * 128],
                        identity=ident[:])
                yoA = sb.tile([128, 4, 128], F32, tag="yo", name=f"yoA_{c}_{b}")
                nc.scalar.mul(yoA[:], cpsA[:], scb[:])
                cpsB = ps.tile([128, 3, 128], F32, tag="xt", bufs=1,
                               name=f"coB_{c}_{b}")
                for j in range(4, 7):
                    nc.tensor.transpose(
                        cpsB[:, j - 4, :], C[:, j, b * 128:(b + 1) * 128],
                        identity=ident[:])
                yoB = sb.tile([128, 3, 128], F32, tag="yo2", name=f"yoB_{c}_{b}")
                nc.scalar.mul(yoB[:], cpsB[:], scb[:])
                orows = slice(c * BC + b * 128, c * BC + (b + 1) * 128)
                for j in range(4):
                    nc.gpsimd.dma_start(
                        out=o_ext[orows, j * 128:(j + 1) * 128],
                        in_=yoA[:, j, :])
                for j in range(4, 6):
                    nc.gpsimd.dma_start(
                        out=o_ext[orows, j * 128:(j + 1) * 128],
                        in_=yoB[:, j - 4, :])
                nc.gpsimd.dma_start(out=o_ext[orows, 768:784],
                                    in_=yoB[:, 2, 0:16])

            # ---- prelude: chunk 0 input + first two F planes ----
            # (x DMAs queue ahead of the big wts DMA on the sync engine)
            for b in range(NBT):
                in_path_b(0, b, dve_quant=(b < 2))
            nc.sync.dma_start(out=wts[:], in_=w_ext[:])
            nc.sync.dma_start(out=scb[:], in_=sc_ext[:].to_broadcast([128, 1]))
            nc.scalar.mul(scb[:], scb[:], 1.0 / T)
            nc.gpsimd.memset(ch[0]["C"][:], 0.0)
            floort_dve(0, 1)
            floort_act(0, 1)
            floort_dve(0, 2)
            floort_act(0, 2)

            # ---- uniform software-pipelined loop over all 64 global steps:
            # V1 runs one step AHEAD (so dh1 never waits on it), layer-4/CNT
            # run two steps BEHIND (slack on the Dh psum WAR). ----
            V1cur = emit_v1(1)
            for s in range(1, NCH * T + 1):
                c, t = (s - 1) // T, (s - 1) % T + 1
                cp, tp = (s - 3) // T, (s - 3) % T + 1   # global step s-2 (lag 2)
                h1, h2, h3b = ch[c]["h1"], ch[c]["h2"], ch[c]["h3b"]
                # vector: F two steps ahead
                floort_sched_dve(c, t)
                # vector: dh1 (V1 for this step finished last step)
                if t == 1:
                    nc.vector._custom_dve(ANT_DROP0, out=h1[:], in0=V1cur[:])
                else:
                    nc.vector._custom_dve(ANT_DROP, out=h1[:], in0=h1[:],
                                          in1=V1cur[:])
                # layer-4 of global step s-2, half 0
                if s >= 3:
                    d0 = d_half(cp, tp, 0)
                    cnt(cp, d0, 0)
                # tensor: V2 = -W2 @ h1 (hi+lo)
                for h in range(2):
                    nc.tensor.matmul(V23ps[:], w2s[h], h1[:],
                                     start=(h == 0), stop=(h == 1))
                if t == 1:
                    nc.vector._custom_dve(ANT_DROP0, out=h2[:], in0=V23ps[:])
                else:
                    nc.vector._custom_dve(ANT_DROP, out=h2[:], in0=h2[:],
                                          in1=V23ps[:])
                if s >= 3:
                    d1 = d_half(cp, tp, 1)
                    cnt(cp, d1, 1)
                # tensor: V3 = -W3 @ h2 (hi+lo)
                for h in range(2):
                    nc.tensor.matmul(V23ps[:], w3s[h], h2[:],
                                     start=(h == 0), stop=(h == 1))
                if t == 1:
                    nc.vector._custom_dve(ANT_DROP0, out=h3b[1][:], in0=V23ps[:])
                else:
                    nc.vector._custom_dve(ANT_DROP, out=h3b[t % 3][:],
                                          in0=h3b[(t - 1) % 3][:], in1=V23ps[:])
                # tensor: V1 for the NEXT global step
                if s < NCH * T:
                    V1cur = emit_v1(s + 1)
                # scalar: F two steps ahead (Act part)
                floort_sched_act(c, t)
                # interleaved chunk bookkeeping (step tails)
                if T - 5 <= t <= T - 2 and c + 1 < NCH:
                    in_path_b(c + 1, t - (T - 5))
                if t == T - 1 and c + 1 < NCH:
                    nc.gpsimd.memset(ch[c + 1]["C"][:], 0.0)
                if 3 <= t <= 6 and c >= 1:
                    out_path_b(c - 1, t - 3)

            # ---- postlude: layer 4 of the last two global steps + out ----
            d0 = d_half(NCH - 1, T - 1, 0)
            cnt(NCH - 1, d0, 0)
            d1 = d_half(NCH - 1, T - 1, 1)
            cnt(NCH - 1, d1, 1)
            d0 = d_half(NCH - 1, T, 0)
            cnt(NCH - 1, d0, 0)
            d1 = d_half(NCH - 1, T, 1)
            out_path_b(NCH - 1, 0)
            out_path_b(NCH - 1, 1)
            cnt(NCH - 1, d1, 1)
            out_path_b(NCH - 1, 2)
            out_path_b(NCH - 1, 3)

    _finalize(nc)
    _CACHE["nc"] = nc
    return nc


def _rne11(x):
    xi = np.asarray(x, np.float32).view(np.uint32).astype(np.uint64)
    half = np.uint64(1 << 11)
    lsb = (xi >> np.uint64(12)) & np.uint64(1)
    q = ((xi + half - np.uint64(1) + lsb) >> np.uint64(12)) << np.uint64(12)
    return np.minimum(q, np.uint64(0xFFFFFFFF)).astype(np.uint32).view(np.float32)


def _prep_inputs(features, W1, W2, W3, W4, out_scale):
    f32 = np.float32
    W1p = np.zeros((H, F), f32); W1p[:, :IN] = W1
    W4p = np.zeros((F, H), f32); W4p[:IN, :] = W4
    W1T = W1p.T.reshape(FT, 128, H).transpose(1, 0, 2).reshape(128, FT * H)
    w123 = np.concatenate([W1T, -W2.T.astype(f32), -W3.T.astype(f32)], axis=1)
    hi = w123.astype(np.float16)
    lo = (w123 - hi.astype(f32)).astype(np.float16)
    w4hi = (-W4p.T).astype(np.float16)
    wts = np.ascontiguousarray(
        np.concatenate([hi, w4hi, lo], axis=1), dtype=np.float16)
    assert wts.shape == (128, 3200)
    sc = np.asarray(out_scale, f32).reshape(1, 1)
    in_maps = []
    for i in range(NCORES):
        in_maps.append({
            "x": np.ascontiguousarray(features[i * BL:(i + 1) * BL], f32),
            "wts": wts,
            "sc": sc,
        })
    return in_maps


def _run(inputs, trace=False):
    nc = _build()
    in_maps = _prep_inputs(**inputs)
    res = run_bass_kernel_spmd(nc, in_maps, core_ids=list(range(NCORES)),
                               trace=trace)
    out = np.concatenate([res.results[i]["out"] for i in range(NCORES)], axis=0)
    return out.astype(np.float32), res


def kernel(**inputs):
    out, _ = _run(inputs, trace=False)
    return out


# revision 15
# speedup vs baseline: 1.1203x; 1.0165x over previous
"""Spiking autoencoder (integrate-and-fire, 16 timesteps) on 8 TRN2 NeuronCores.

Data-parallel: batch 16384 split as 8 x 2048. Per core, a fully fused
Bass/Tile kernel:

  - features are PE-transposed to feature-major [896(pad), B] layout and
    quantized to integer levels G = rne(16*x) (one fused DVE op).
  - Layer-0 spike counts have the closed form F_t = floor(G*t/16): computed
    DIRECTLY per step (no recurrence) with a fused floor-by-2^23 op, split
    between the Scalar(Act) engine (tiles 0:4, two affine ops) and the DVE
    (tiles 4:7, one fused op). F_t feeds V1_t = W1 @ F_t.
  - Layers 1-3 keep an integer spike-count state h = -#spikes:
        s_t = [h + V >= 1], h -= s_t
    and layers 2-4 rebuild potentials fresh as -W @ h (exact), so spikes
    are never materialized. Layer 4 keeps only the output count:
    C += [V4 - C >= 1].
  - W1/W2/W3 matmuls run as float32r split hi=rne11(W), lo=W-hi (two
    accumulating fp32r matmuls = fp32 precision). W4 uses hi only (12-bit
    weights) - validated to keep rel err ~8e-3, and halves decoder matmuls.
  - Layer-4 work is split into 4 feature/batch groups pipelined against
    the DVE count op; the whole step is scheduled so tensor and vector
    engines overlap (layer-4 of step t-1 fills layer-2/3 gaps of step t).

Custom DVE ops (fused, registered at import): ANT_DROP, ANT_COUNT,
ANT_DROP0, ANT_ROUND16, ANT_FLOORT.
"""
import sys
import copy
import itertools

sys.path.insert(0, "/opt/trn_rl_repo")

import numpy as np

# ----------------------------------------------------------------------------
# Custom DVE op registration
# ----------------------------------------------------------------------------
import concourse.dve_ops as dve_ops
from concourse.dve_ops import DveOp
from concourse.dve_spec import (
    Spec, Src0, Src1, One, Zero, C0, C1, C2, lower, _has_src1 as has_src1,
)
from concourse.dve_table_gen import DveOpSpec

_F = np.float32


def _register(name, spec):
    if name in dve_ops._SUB_OPCODE_FOR_NAME:
        return next(op for op in dve_ops.OPS if op.name == name)
    shas = {}
    for ver in ("v3", "v4"):
        s = DveOpSpec(name=name, opcode=0, uops=lower(spec, ver=ver),
                      rd1_en=has_src1(spec))
        shas[ver] = s.sha(ver)
    op = DveOp(name, spec, subdim=False, uops_sha=shas)
    dve_ops.OPS.append(op)
    dve_ops._SUB_OPCODE_FOR_NAME[name] = (
        dve_ops._CUSTOM_DVE_ROW_BASE + len(dve_ops.OPS) - 1)
    dve_ops.CUSTOM_DVE_SPECS[name] = spec
    assert dve_ops._SUB_OPCODE_FOR_NAME[name] < 0x20
    return op


# h' = h - ((h + V) >= 1)   (state update when V is cumulative)
ANT_DROP = _register("ANT_DROP", Spec(
    body=Src0 - ((Src0 + Src1) >= One),
    reference=lambda in0, in1, s0, s1, imm2:
        in0 - ((in0 + in1) >= 1.0).astype(_F)))

# C' = C + ((V - C) >= 1)
ANT_COUNT = _register("ANT_COUNT", Spec(
    body=Src0 + ((Src1 - Src0) >= One),
    reference=lambda in0, in1, s0, s1, imm2:
        in0 + ((in1 - in0) >= 1.0).astype(_F)))

# h' = -(V >= 1)   (first cumulative-drop step from zero state)
ANT_DROP0 = _register("ANT_DROP0", Spec(
    body=Zero - (Src0 >= One),
    reference=lambda in0, s0, s1, imm2: -((in0 >= 1.0).astype(_F))))

# out = ((x*C0 + C1) - C1) * C2  -> round-to-nearest-even via the 2^23 trick
ANT_ROUND16 = _register("ANT_ROUND16", Spec(
    body=((Src0 * C0 + C1) - C1) * C2,
    reference=lambda in0, s0, s1, imm2: (
        (np.float32(in0 * np.float32(s0)) + np.float32(s1))
        - np.float32(s1)) * np.float32(imm2)))

# out = ((x*C0 - C2) + C1) - C1 ; with C2 = 15/32, C1 = 3*2^22 this is
# floor(x*C0) exactly, for x*C0 a multiple of 1/16 in [0, 16]: x*C0 - 15/32
# lands strictly inside (floor-1/2, floor+1/2), and adding 3*2^22 rounds to
# integer (ulp 1) without ever leaving the [2^23, 2^24) binade.
ANT_FLOORT = _register("ANT_FLOORT", Spec(
    body=((Src0 * C0 - C2) + C1) - C1,
    reference=lambda in0, s0, s1, imm2: (
        (np.float32(np.float32(in0 * np.float32(s0)) - np.float32(imm2))
         + np.float32(s1)) - np.float32(s1))))

# ----------------------------------------------------------------------------
# Walrus-compat fixes (this container's neuronxcc rejects >1 sem-wait on
# many instruction structs and any wait on InstDrain; raw Bass also skips
# the pass that packs extended-inst ISA bytes).
# ----------------------------------------------------------------------------
from concourse import bass, mybir
from concourse.tile import TileContext
from concourse.vector_clock import ScopedClock
from concourse.bass_utils import run_bass_kernel_spmd
from concourse.masks import make_identity

_ctr = itertools.count()


def _build_wait_templates():
    nc = bass.Bass(target_bir_lowering=False)
    out = {}
    with nc.Block() as block, nc.semaphore("s") as s:
        for eng_name in ("sync", "vector", "scalar", "gpsimd", "tensor"):
            def _mk(e, _out=out):
                i = e.wait_ge(s, 0)
                _out[i.ins.engine] = i.ins
            getattr(block, eng_name)(_mk)
    return out


_WAIT_TEMPLATES = _build_wait_templates()


def _mk_wait(engine, w):
    wi = copy.deepcopy(_WAIT_TEMPLATES[engine])
    wi.name = f"I-waitsplit-{next(_ctr)}"
    si = wi.sync_info
    si.on_wait.clear()
    si.on_wait.append(w)
    return wi


def _fix_waits(nc, limit=1):
    n = 0
    for bb in nc.main_func.blocks:
        il = bb.instructions
        i = 0
        while i < len(il):
            ins = il[i]
            lim = 0 if type(ins).__name__ == "InstDrain" else limit
            si = ins.sync_info
            waits = list(si.on_wait) if (si and si.on_wait) else []
            if type(ins).__name__ != "InstEventSemaphore" and len(waits) > lim:
                keep, extra = waits[:lim], waits[lim:]
                si.on_wait.clear()
                for w in keep:
                    si.on_wait.append(w)
                for j, w in enumerate(extra):
                    il.insert(i + j, _mk_wait(ins.engine, w))
                i += len(extra)
                n += 1
            i += 1
    return n


def _finalize(nc):
    from concourse.library_overlay import lower_extended_insts
    lower_extended_insts(nc)
    return _fix_waits(nc)


def _patched_drain_and_barrier(self, tick_clock, wait_clock):
    nc = self.nc
    probe = nc.sync.nop()
    wait_clock.add_sem_waits(probe.ins, ScopedClock({None: tick_clock.global_clock}))
    si = probe.ins.sync_info
    waits = list(si.on_wait or []) if si is not None else []
    if si is not None and si.on_wait:
        si.on_wait.clear()
    handles = list(self.sems.allocated().values())
    by_name = {getattr(h, "name", None): h for h in handles}
    for w in waits:
        nc.sync.wait_ge(by_name[w.ant_name], w.wait_value)
    nc.sync.drain()
    nc.all_engine_barrier()
    popped = nc._tile_sem_poison_stack.pop()
    assert popped is self._sem_poison
    nc.clear_and_free_semaphores(handles)
    nc.all_engine_barrier()


TileContext._drain_and_barrier = _patched_drain_and_barrier

# ----------------------------------------------------------------------------
# Kernel build
# ----------------------------------------------------------------------------
F32 = mybir.dt.float32
F32R = mybir.dt.float32r
F16 = mybir.dt.float16

NCORES = 8
B = 16384
BL = B // NCORES          # 2048 per core
IN = 784
H = 128
T = 16
FT = 7                    # feature tiles
F = FT * 128              # 896 padded
BC = 512                  # batch chunk (psum-bank limited)
NCH = BL // BC            # 4 chunks
NBT = BC // 128           # 4 batch subtiles per chunk
HB = BC // 2              # 256: layer-4 half-batch group width

C23 = float(2 ** 23)
M32 = float(3 * 2 ** 22)   # 12582912, exact
OFF = 15.0 / 32.0          # 0.46875, exact

_CACHE = {}

IDENT = mybir.ActivationFunctionType.Identity
ACOPY = mybir.ActivationFunctionType.Copy
ALU = mybir.AluOpType


def _build():
    if "nc" in _CACHE:
        return _CACHE["nc"]
    nc = bass.Bass(target_bir_lowering=False)
    x_ext = nc.declare_dram_parameter("x", [BL, IN], F32, isOutput=False)
    w_ext = nc.declare_dram_parameter("wts", [128, 3200], F16, isOutput=False)
    sc_ext = nc.declare_dram_parameter("sc", [1, 1], F32, isOutput=False)
    o_ext = nc.declare_dram_parameter("out", [BL, IN], F32, isOutput=True)

    with TileContext(nc) as tc:
        with (tc.tile_pool(name="const", bufs=1) as constp,
              tc.tile_pool(name="sb", bufs=2) as sb,
              tc.tile_pool(name="st", bufs=1) as st,
              tc.tile_pool(name="st2", bufs=2) as st2,
              tc.tile_pool(name="ps", bufs=1, space="PSUM") as ps):

            wts = constp.tile([128, 3200], F16, tag="wts")
            ident = constp.tile([128, 128], F32, tag="ident")
            scb = constp.tile([128, 1], F32, tag="scb")
            make_identity(nc, ident[:])

            # wts layout: [ hi W1T 0:896 | hi W2 896:1024 | hi W3 1024:1152 |
            #               W4T(hi only) 1152:2048 |
            #               lo W1T 2048:2944 | lo W2 2944:3072 | lo W3 3072:3200 ]
            w1s = ([wts[:, k * 128:(k + 1) * 128] for k in range(FT)],
                   [wts[:, 2048 + k * 128:2048 + (k + 1) * 128] for k in range(FT)])
            w2s = (wts[:, 896:1024], wts[:, 2944:3072])
            w3s = (wts[:, 1024:1152], wts[:, 3072:3200])
            w4s = [wts[:, 1152 + j * 128:1152 + (j + 1) * 128] for j in range(FT)]

            ya = st.tile([128, 4, BC], F32, tag="ya", name="ya")
            yv = st.tile([128, 3, BC], F16, tag="yv", name="yv")

            # per-chunk state tiles (tag rotation order == creation order)
            ch = []
            for c in range(NCH):
                ch.append({
                    "G": st2.tile([128, FT, BC], F16, tag="G", name=f"G_{c}"),
                    "Fr": [st.tile([128, FT, BC], F16, tag=f"Fr{i}",
                                   name=f"Fr{i}_{c}") for i in range(3)],
                    "h1": st.tile([128, BC], F16, tag="h1", name=f"h1_{c}"),
                    "h2": st.tile([128, BC], F16, tag="h2", name=f"h2_{c}"),
                    "h3b": [st2.tile([128, BC], F16, tag=f"h3{i}",
                                     name=f"h3{i}_{c}") for i in range(3)],
                    "C": st2.tile([128, FT, BC], F32, tag="C", name=f"C_{c}"),
                })

            V23ps = ps.tile([128, BC], F32, tag="V23", name="V23ps")
            V1ps = ps.tile([128, BC], F32, tag="V1", name="V1ps")

            def emit_v1(s2):
                """V1 block for global step s2."""
                c2, t2 = (s2 - 1) // T, (s2 - 1) % T + 1
                Ft = ch[c2]["Fr"][t2 % 3] if t2 < T else ch[c2]["G"]
                for k in range(FT):
                    for h in range(2):
                        nc.tensor.matmul(V1ps[:], w1s[h][k], Ft[:, k, :],
                                         start=(k == 0 and h == 0),
                                         stop=(k == FT - 1 and h == 1))
                return V1ps

            def in_path_b(c, b, dve_quant=False):
                """DMA + transpose + quantize one 128-row batch subtile."""
                G = ch[c]["G"]
                xt = sb.tile([128, IN], F32, tag="x", name=f"x_{c}_{b}")
                nc.sync.dma_start(
                    out=xt[:],
                    in_=x_ext[c * BC + b * 128:c * BC + (b + 1) * 128, :])
                xpsB = ps.tile([128, 3, 128], F32, tag="xt", bufs=2,
                               name=f"xpsB_{c}_{b}")
                nc.vector.memset(xpsB[:, 2, :], 0.0)
                for j in range(4, 6):
                    nc.tensor.transpose(
                        xpsB[:, j - 4, :], xt[:, j * 128:(j + 1) * 128],
                        identity=ident[:])
                nc.tensor.transpose(
                    xpsB[0:16, 2, :], xt[:, 768:784], identity=ident[:])
                yqB = sb.tile([128, 3, 128], F32, tag="yq2", name=f"yqB_{c}_{b}")
                if dve_quant:
                    nc.vector.tensor_scalar(out=yqB[:], in0=xpsB[:, :, :],
                                            scalar1=16.0, scalar2=M32,
                                            op0=ALU.mult, op1=ALU.add)
                    nc.vector.tensor_scalar(
                        out=G[:, 4:7, b * 128:(b + 1) * 128], in0=yqB[:],
                        scalar1=M32, scalar2=None, op0=ALU.subtract)
                else:
                    nc.scalar.activation(yqB[:], xpsB[:, :, :], ACOPY,
                                         bias=M32, scale=16.0)
                    nc.scalar.activation(G[:, 4:7, b * 128:(b + 1) * 128],
                                         yqB[:], ACOPY, bias=-M32, scale=1.0)
                xpsA = ps.tile([128, 4, 128], F32, tag="xt", bufs=2,
                               name=f"xpsA_{c}_{b}")
                for j in range(4):
                    nc.tensor.transpose(
                        xpsA[:, j, :], xt[:, j * 128:(j + 1) * 128],
                        identity=ident[:])
                yqA = sb.tile([128, 4, 128], F32, tag="yq", name=f"yqA_{c}_{b}")
                if dve_quant:
                    nc.vector.tensor_scalar(out=yqA[:], in0=xpsA[:, :, :],
                                            scalar1=16.0, scalar2=M32,
                                            op0=ALU.mult, op1=ALU.add)
                    nc.vector.tensor_scalar(
                        out=G[:, 0:4, b * 128:(b + 1) * 128], in0=yqA[:],
                        scalar1=M32, scalar2=None, op0=ALU.subtract)
                else:
                    nc.scalar.activation(yqA[:], xpsA[:, :, :], ACOPY,
                                         bias=M32, scale=16.0)
                    nc.scalar.activation(G[:, 0:4, b * 128:(b + 1) * 128],
                                         yqA[:], ACOPY, bias=-M32, scale=1.0)

            def floort_dve(c, t):
                """F_t tiles 4:7 on DVE (two builtin tensor_scalar ops -
                builtins hit the fast DVE perf modes, customs do not)."""
                Fo = ch[c]["Fr"][t % 3]
                nc.vector.tensor_scalar(
                    out=yv[:], in0=ch[c]["G"][:, 4:7, :],
                    scalar1=float(t) / 16.0, scalar2=OFF,
                    op0=ALU.mult, op1=ALU.subtract)
                nc.vector.tensor_scalar(
                    out=Fo[:, 4:7, :], in0=yv[:],
                    scalar1=M32, scalar2=M32, op0=ALU.add, op1=ALU.subtract)

            def floort_act(c, t):
                """F_t tiles 0:4 on Act (three exact affine Copy ops)."""
                Fo = ch[c]["Fr"][t % 3]
                nc.scalar.activation(ya[:], ch[c]["G"][:, 0:4, :], ACOPY,
                                     bias=-OFF, scale=float(t) / 16.0)
                nc.scalar.activation(ya[:], ya[:], ACOPY, bias=M32, scale=1.0)
                nc.scalar.activation(Fo[:, 0:4, :], ya[:], ACOPY,
                                     bias=-M32, scale=1.0)

            def floort_sched_dve(c, t):
                if t <= T - 3:
                    floort_dve(c, t + 2)
                elif t == T - 1 and c + 1 < NCH:
                    floort_dve(c + 1, 1)
                elif t == T and c + 1 < NCH:
                    floort_dve(c + 1, 2)

            def floort_sched_act(c, t):
                if t <= T - 3:
                    floort_act(c, t + 2)
                elif t == T - 1 and c + 1 < NCH:
                    floort_act(c + 1, 1)
                elif t == T and c + 1 < NCH:
                    floort_act(c + 1, 2)

            def d_half(c, tp, half):
                """Layer-4 potentials for (chunk c, step tp), one batch half."""
                h3 = ch[c]["h3b"][tp % 3]
                lo = half * HB
                dps = ps.tile([128, FT, HB], F32, tag="Dh", bufs=1,
                              name=f"Dh_{c}_{tp}_{half}")
                for j in range(FT):
                    nc.tensor.matmul(dps[:, j, :], w4s[j], h3[:, lo:lo + HB],
                                     start=True, stop=True)
                return dps

            def cnt(c, dps, half):
                C = ch[c]["C"]
                lo = half * HB
                nc.vector._custom_dve(
                    ANT_COUNT, out=C[:, :, lo:lo + HB],
                    in0=C[:, :, lo:lo + HB], in1=dps[:])

            def out_path_b(c, b):
                C = ch[c]["C"]
                cpsA = ps.tile([128, 4, 128], F32, tag="xt", bufs=2,
                               name=f"coA_{c}_{b}")
                for j in range(4):
                    nc.tensor.transpose(
                        cpsA[:, j, :], C[:, j, b * 128:(b + 1) * 128],
                        identity=ident[:])
                yoA = sb.tile([128, 4, 128], F32, tag="yo", name=f"yoA_{c}_{b}")
                nc.scalar.mul(yoA[:], cpsA[:], scb[:])
                cpsB = ps.tile([128, 3, 128], F32, tag="xt", bufs=2,
                               name=f"coB_{c}_{b}")
                for j in range(4, 7):
                    nc.tensor.transpose(
                        cpsB[:, j - 4, :], C[:, j, b * 128:(b + 1) * 128],
                        identity=ident[:])
                yoB = sb.tile([128, 3, 128], F32, tag="yo2", name=f"yoB_{c}_{b}")
                nc.scalar.mul(yoB[:], cpsB[:], scb[:])
                orows = slice(c * BC + b * 128, c * BC + (b + 1) * 128)
                for j in range(4):
                    nc.gpsimd.dma_start(
                        out=o_ext[orows, j * 128:(j + 1) * 128],
                        in_=yoA[:, j, :])
                for j in range(4, 6):
                    nc.gpsimd.dma_start(
                        out=o_ext[orows, j * 128:(j + 1) * 128],
                        in_=yoB[:, j - 4, :])
                nc.gpsimd.dma_start(out=o_ext[orows, 768:784],
                                    in_=yoB[:, 2, 0:16])

            # ---- prelude: chunk 0 input + first two F planes ----
            # (x DMAs queue ahead of the big wts DMA on the sync engine)
            for b in range(NBT):
                in_path_b(0, b, dve_quant=(b < 2))
            nc.sync.dma_start(out=wts[:], in_=w_ext[:])
            nc.sync.dma_start(out=scb[:], in_=sc_ext[:].to_broadcast([128, 1]))
            nc.scalar.mul(scb[:], scb[:], 1.0 / T)
            nc.gpsimd.memset(ch[0]["C"][:], 0.0)
            floort_dve(0, 1)
            floort_act(0, 1)
            floort_dve(0, 2)
            floort_act(0, 2)

            # ---- uniform software-pipelined loop over all 64 global steps:
            # layer-4/CNT run two steps BEHIND (slack on the Dh psum WAR). ----
            for s in range(1, NCH * T + 1):
                c, t = (s - 1) // T, (s - 1) % T + 1
                cp, tp = (s - 3) // T, (s - 3) % T + 1   # global step s-2 (lag 2)
                h1, h2, h3b = ch[c]["h1"], ch[c]["h2"], ch[c]["h3b"]
                # tensor: V1 for this step
                V1cur = emit_v1(s)
                # vector: F two steps ahead
                floort_sched_dve(c, t)
                # vector: dh1 (V1 for this step finished last step)
                if t == 1:
                    nc.vector._custom_dve(ANT_DROP0, out=h1[:], in0=V1cur[:])
                else:
                    nc.vector._custom_dve(ANT_DROP, out=h1[:], in0=h1[:],
                                          in1=V1cur[:])
                # layer-4 of global step s-2, half 0
                if s >= 3:
                    d0 = d_half(cp, tp, 0)
                    cnt(cp, d0, 0)
                # tensor: V2 = -W2 @ h1 (hi+lo)
                for h in range(2):
                    nc.tensor.matmul(V23ps[:], w2s[h], h1[:],
                                     start=(h == 0), stop=(h == 1))
                if t == 1:
                    nc.vector._custom_dve(ANT_DROP0, out=h2[:], in0=V23ps[:])
                else:
                    nc.vector._custom_dve(ANT_DROP, out=h2[:], in0=h2[:],
                                          in1=V23ps[:])
                if s >= 3:
                    d1 = d_half(cp, tp, 1)
                    cnt(cp, d1, 1)
                # tensor: V3 = -W3 @ h2 (hi+lo)
                for h in range(2):
                    nc.tensor.matmul(V23ps[:], w3s[h], h2[:],
                                     start=(h == 0), stop=(h == 1))
                if t == 1:
                    nc.vector._custom_dve(ANT_DROP0, out=h3b[1][:], in0=V23ps[:])
                else:
                    nc.vector._custom_dve(ANT_DROP, out=h3b[t % 3][:],
                                          in0=h3b[(t - 1) % 3][:], in1=V23ps[:])
                # scalar: F two steps ahead (Act part)
                floort_sched_act(c, t)
                # interleaved chunk bookkeeping (step tails)
                if T - 5 <= t <= T - 2 and c + 1 < NCH:
                    in_path_b(c + 1, t - (T - 5))
                if t == T - 1 and c + 1 < NCH:
                    nc.gpsimd.memset(ch[c + 1]["C"][:], 0.0)
                if 3 <= t <= 6 and c >= 1:
                    out_path_b(c - 1, t - 3)

            # ---- postlude: layer 4 of the last two global steps + out ----
            d0 = d_half(NCH - 1, T - 1, 0)
            cnt(NCH - 1, d0, 0)
            d1 = d_half(NCH - 1, T - 1, 1)
            cnt(NCH - 1, d1, 1)
            d0 = d_half(NCH - 1, T, 0)
            cnt(NCH - 1, d0, 0)
            d1 = d_half(NCH - 1, T, 1)
            out_path_b(NCH - 1, 0)
            out_path_b(NCH - 1, 1)
            cnt(NCH - 1, d1, 1)
            out_path_b(NCH - 1, 2)
            out_path_b(NCH - 1, 3)

    _finalize(nc)
    _CACHE["nc"] = nc
    return nc


def _rne11(x):
    xi = np.asarray(x, np.float32).view(np.uint32).astype(np.uint64)
    half = np.uint64(1 << 11)
    lsb = (xi >> np.uint64(12)) & np.uint64(1)
    q = ((xi + half - np.uint64(1) + lsb) >> np.uint64(12)) << np.uint64(12)
    return np.minimum(q, np.uint64(0xFFFFFFFF)).astype(np.uint32).view(np.float32)


def _prep_inputs(features, W1, W2, W3, W4, out_scale):
    f32 = np.float32
    W1p = np.zeros((H, F), f32); W1p[:, :IN] = W1
    W4p = np.zeros((F, H), f32); W4p[:IN, :] = W4
    W1T = W1p.T.reshape(FT, 128, H).transpose(1, 0, 2).reshape(128, FT * H)
    w123 = np.concatenate([W1T, -W2.T.astype(f32), -W3.T.astype(f32)], axis=1)
    hi = w123.astype(np.float16)
    lo = (w123 - hi.astype(f32)).astype(np.float16)
    w4hi = (-W4p.T).astype(np.float16)
    wts = np.ascontiguousarray(
        np.concatenate([hi, w4hi, lo], axis=1), dtype=np.float16)
    assert wts.shape == (128, 3200)
    sc = np.asarray(out_scale, f32).reshape(1, 1)
    in_maps = []
    for i in range(NCORES):
        in_maps.append({
            "x": np.ascontiguousarray(features[i * BL:(i + 1) * BL], f32),
            "wts": wts,
            "sc": sc,
        })
    return in_maps


def _run(inputs, trace=False):
    nc = _build()
    in_maps = _prep_inputs(**inputs)
    res = run_bass_kernel_spmd(nc, in_maps, core_ids=list(range(NCORES)),
                               trace=trace)
    out = np.concatenate([res.results[i]["out"] for i in range(NCORES)], axis=0)
    return out.astype(np.float32), res


def kernel(**inputs):
    out, _ = _run(inputs, trace=False)
    return out
